# revision 25
# baseline (speedup 1.0000x reference)
"""CorrelationLayer1D Trainium2 Bass kernel (fp16 pipeline).

Computes out[b, d, h, w] = sum_c x_1[b,c,h,w] * x2p[b,c,h,w+d] for d in [0, 41),
where x2p is x_2 width-padded by (8, 32).  Inputs [4,128,160,320] f32.

Sharding: data-parallel over H = 160 = 8*20 (correlation runs along W only, so
H-sharding needs no halo).  Host converts inputs fp32 -> fp16 (rel-err budget
2e-2 >> fp16's ~1e-3) and converts the fp16 device output back to fp32.

Per core, rows are processed in chunks of HC=10 h-rows, software-pipelined
with lag 2 (the transpose/output stage of chunk N is emitted after the gram
stage of chunk N+2 so the DRAM round-trip latency hides under gram work):

Stage 1 (per chunk):
  x1/x2 load as contiguous [C, HC*W] fp16 tiles on the SP HWDGE ring, which
  carries nothing that waits on compute, so prefetch is never head-of-line
  blocked.  Per (h, w-block) the PE computes a block-diagonal Gram into
  PSUM: two M=64 matmuls (lo/hi) per 128-wide block land as a compacted
  [128, 104] band tile; 4 h-rows of one block share a PSUM bank
  ([128, 416] f32).  Edge blocks use narrowed windows (x2 is unpadded in
  SBUF) and the uncomputed strips are zeroed in the atlas by GpSimd.
  VectorE drains PSUM -> per-kblk fp16 atlas [128, HC*104].  GpSimd SWDGE
  stores the atlas to DRAM scratch (row pitch padded +63).

Stage 2 (lag 2 behind):
  Full-row skewed reloads (GpSimd SWDGE): partition i reads its whole
  HC*104 row shifted by i elements (DRAM-side AP partition step =
  pitch+1), so each partition is one contiguous descriptor and row h's
  41-wide band lands at [h*104, h*104+41).  SBUF-side per-partition skews
  are not expressible, which is why the round-trip exists.  PE transposes
  bands to [d, w] via matmul with an fp16 identity; rows h and h+HC/2
  share one PSUM bank at partition bases 0/64; ScalarE drains each
  [105, 320] bank into a [105, HC/2*W] tile whose halves store as two
  half-chunk output DMAs on the SP ring (the ACT HWDGE queue is served by
  a single SDMA engine, so bulk data never goes there).
"""
import sys

import numpy as np

try:
    import concourse.bass as bass  # noqa: F401
except ImportError:
    sys.path.insert(0, "/opt/trn_rl_repo")

import concourse.bass as bass
import concourse.tile as tile
from concourse import bacc, masks, mybir
from concourse.ap import AP
from concourse.bass_utils import run_bass_kernel_spmd

MAX_DISP = 40
D = MAX_DISP + 1  # 41 displacements
PAD_L = 8
B, C, H, W = 4, 128, 160, 320
N_CORES = 8
HS = H // N_CORES  # 20 h-rows per core
WBLOCKS = [(0, 128), (128, 128), (256, 64)]
GW = 64 + MAX_DISP  # compacted gram width per 64-block: 104

F32 = mybir.dt.float32
F16 = mybir.dt.float16

# (kblk, j0, N) for the lo-half matmul of each w-block: the window starts at
# x2p column w0+j0 and spans N columns, all of which exist in the unpadded x2
# row.  Gram columns outside [j0, j0+N) are zeroed in the atlas.
LO_WIN = {0: (8, 96), 1: (0, 104), 2: (0, 72)}


def build_kernel(b_dim=B, hs=HS, hc=None):
    if hc is None:
        hc = 20 if hs % 20 == 0 else hs
    assert hs % hc == 0 and hc % 2 == 0
    nchunks = hs // hc
    hgroups = [(s, min(4, hc - s)) for s in range(0, hc, 4)]

    nc = bacc.Bacc("TRN2", target_bir_lowering=False, debug=False)
    x1e = nc.declare_dram_parameter("x1", [b_dim, C, hs, W], F16, isOutput=False)
    x2e = nc.declare_dram_parameter("x2", [b_dim, C, hs, W], F16, isOutput=False)
    oute = nc.declare_dram_parameter("out", [b_dim, D, hs, W], F16, isOutput=True)

    with tile.TileContext(nc) as tc:
        with (
            tc.tile_pool(name="const", bufs=1) as const_pool,
            tc.tile_pool(name="xin", bufs=2) as xin_pool,
            tc.tile_pool(name="atlas", bufs=3) as atlas_pool,
            tc.tile_pool(name="sbig", bufs=3) as sbig_pool,
            tc.tile_pool(name="asm", bufs=3) as asm_pool,
            tc.tile_pool(name="psum_g", bufs=2, space="PSUM") as psum_g,
            tc.tile_pool(name="psum_t", bufs=2, space="PSUM") as psum_t,
            tc.tile_pool(name="scratch", bufs=3, space="DRAM") as scratch_pool,
        ):
            ident = const_pool.tile([128, 128], F16)
            masks.make_identity(nc, ident[:])

            def emit_stage1(b, ci):
                h0 = ci * hc
                x1b = xin_pool.tile([C, hc * W], F16, tag="x1b", name=f"x1b_{b}_{ci}")
                nc.sync.dma_start(
                    x1b[:].rearrange("p (h w) -> p h w", w=W),
                    x1e[b, :, h0 : h0 + hc, :],
                )
                x2b = xin_pool.tile([C, hc * W], F16, tag="x2b", name=f"x2b_{b}_{ci}")
                nc.sync.dma_start(
                    x2b[:].rearrange("p (h w) -> p h w", w=W),
                    x2e[b, :, h0 : h0 + hc, :],
                )

                atlas = [
                    atlas_pool.tile([128, hc * GW], F16, tag="atl0", name=f"atl0_{b}_{ci}"),
                    atlas_pool.tile([128, hc * GW], F16, tag="atl1", name=f"atl1_{b}_{ci}"),
                    atlas_pool.tile([64, hc * GW], F16, tag="atl2", name=f"atl2_{b}_{ci}"),
                ]

                # Gram phase: 4 h-rows of one kblk share a PSUM bank.
                ncopy = 0
                for s0, ng in hgroups:
                    pg = [
                        psum_g.tile([128, 4 * GW], F32, tag="pg0", name=f"pg0_{b}_{ci}_{s0}"),
                        psum_g.tile([128, 4 * GW], F32, tag="pg1", name=f"pg1_{b}_{ci}_{s0}"),
                        psum_g.tile([64, 4 * GW], F32, tag="pg2", name=f"pg2_{b}_{ci}_{s0}"),
                    ]
                    for hi in range(ng):
                        hh = s0 + hi
                        o1 = hh * W
                        for kblk, (w0, wb) in enumerate(WBLOCKS):
                            j0, nlo = LO_WIN[kblk]
                            c0 = hi * GW
                            nc.tensor.matmul(
                                pg[kblk][0:64, c0 + j0 : c0 + j0 + nlo],
                                x1b[:, o1 + w0 : o1 + w0 + 64],
                                x2b[:, o1 + w0 + j0 - PAD_L : o1 + w0 + j0 - PAD_L + nlo],
                                start=True,
                                stop=True,
                                tile_position=(0, 0),
                            )
                            if wb == 128:
                                nc.tensor.matmul(
                                    pg[kblk][64:128, c0 : c0 + GW],
                                    x1b[:, o1 + w0 + 64 : o1 + w0 + 128],
                                    x2b[:, o1 + w0 + 56 : o1 + w0 + 56 + GW],
                                    start=True,
                                    stop=True,
                                    tile_position=(0, 64),
                                )
                    # Drain the 3 banks to the fp16 atlas (DVE only: the ACT
                    # and SP sequencers stay free to issue DMAs without
                    # queueing copies behind semaphore waits).
                    for kblk, (w0, wb) in enumerate(WBLOCKS):
                        dst = atlas[kblk][0:wb, s0 * GW : (s0 + ng) * GW]
                        srcp = pg[kblk][0:wb, 0 : ng * GW]
                        nc.vector.tensor_copy(dst, srcp)
                        ncopy += 1

                # Zero the gram strips the narrowed edge matmuls skipped.
                a0 = atlas[0][0:64, :].rearrange("p (h j) -> p h j", j=GW)
                nc.gpsimd.memset(a0[:, :, 0:8], 0.0)
                a2 = atlas[2][0:64, :].rearrange("p (h j) -> p h j", j=GW)
                nc.gpsimd.memset(a2[:, :, 72:GW], 0.0)

                # Scratch round-trip: rectangular store (row pitch padded by
                # 63 so the skewed read stays in-bounds), then a full-row
                # skewed reload: partition i reads the whole hc*104 row
                # shifted by i elements, so each partition is one contiguous
                # descriptor and the band of row h sits at [h*104, h*104+41).
                RW = hc * GW
                R = RW + 63
                sbig = [
                    sbig_pool.tile([128, RW], F16, tag="sb0", name=f"sb0_{b}_{ci}"),
                    sbig_pool.tile([128, RW], F16, tag="sb1", name=f"sb1_{b}_{ci}"),
                    sbig_pool.tile([64, RW], F16, tag="sb2", name=f"sb2_{b}_{ci}"),
                ]
                scrs = []
                for kblk, (w0, wb) in enumerate(WBLOCKS):
                    scr = scratch_pool.tile(
                        [wb, R], F16, tag=f"scr{kblk}", name=f"scr{kblk}_{b}_{ci}"
                    )
                    nc.gpsimd.dma_start(scr[:, 0:RW], atlas[kblk][0:wb, :])
                    scrs.append(scr)
                return b, ci, sbig, scrs

            def emit_stage2(state):
                b, ci, sbig, scrs = state
                h0 = ci * hc
                hc2 = hc // 2
                RW = hc * GW
                R = RW + 63
                # Full-row skewed reloads: partition step = pitch+1.
                for kblk, (w0, wb) in enumerate(WBLOCKS):
                    scr_ap = scrs[kblk][:]
                    for half in range(wb // 64):
                        diag = AP(
                            tensor=scr_ap.tensor,
                            offset=scr_ap.offset + half * 64 * R,
                            ap=[[R + 1, 64], [1, RW]],
                        )
                        nc.gpsimd.dma_start(
                            sbig[kblk][half * 64 : (half + 1) * 64, :], diag
                        )
                # Transpose bands to [d, w]: rows h and h+hc/2 share one PSUM
                # bank at partition bases 0 and 64, drained by a single copy
                # into a [105, hc/2*W] tile whose halves store as two
                # contiguous half-chunk output DMAs.
                abatch = asm_pool.tile(
                    [64 + D, hc2 * W], F16, tag="ab", name=f"ab_{b}_{ci}"
                )
                for p0 in range(hc2):
                    pt = psum_t.tile(
                        [64 + D, W], F32, tag="pt", name=f"pt_{b}_{ci}_{p0}"
                    )
                    for kblk, (w0, wb) in enumerate(WBLOCKS):
                        nc.tensor.matmul(
                            pt[0:D, w0 : w0 + wb],
                            sbig[kblk][0:wb, p0 * GW : p0 * GW + D],
                            ident[0:wb, 0:wb],
                            start=True,
                            stop=True,
                            tile_position=(0, 0),
                        )
                        nc.tensor.matmul(
                            pt[64 : 64 + D, w0 : w0 + wb],
                            sbig[kblk][0:wb, (p0 + hc2) * GW : (p0 + hc2) * GW + D],
                            ident[0:wb, 0:wb],
                            start=True,
                            stop=True,
                            tile_position=(0, 64),
                        )
                    nc.scalar.copy(abatch[:, p0 * W : (p0 + 1) * W], pt[:])

                nc.sync.dma_start(
                    oute[b, :, h0 : h0 + hc2, :],
                    abatch[0:D, :].rearrange("d (h w) -> d h w", w=W),
                )
                nc.sync.dma_start(
                    oute[b, :, h0 + hc2 : h0 + hc, :],
                    abatch[64 : 64 + D, :].rearrange("d (h w) -> d h w", w=W),
                )

            # Software pipeline, lag 2: stage2 of chunk N runs after stage1
            # of chunk N+2, so the ~13us scratch round-trip latency of chunk
            # N hides under two chunks of gram work.
            LAG = 2
            chunks = [(b, ci) for b in range(b_dim) for ci in range(nchunks)]
            pending = []
            for b, ci in chunks:
                pending.append(emit_stage1(b, ci))
                if len(pending) > LAG:
                    emit_stage2(pending.pop(0))
            for st in pending:
                emit_stage2(st)

    nc.finalize()
    return nc


_compiled = {}


def _get_kernel(b_dim, hs):
    key = (b_dim, hs)
    if key not in _compiled:
        _compiled[key] = build_kernel(b_dim, hs)
    return _compiled[key]


def kernel(x_1: np.ndarray, x_2: np.ndarray) -> np.ndarray:
    assert x_1.shape == (B, C, H, W) and x_2.shape == (B, C, H, W)
    x_1 = np.ascontiguousarray(x_1, dtype=np.float16)
    x_2 = np.ascontiguousarray(x_2, dtype=np.float16)
    nc = _get_kernel(B, HS)
    in_maps = [
        {
            "x1": np.ascontiguousarray(x_1[:, :, i * HS : (i + 1) * HS, :]),
            "x2": np.ascontiguousarray(x_2[:, :, i * HS : (i + 1) * HS, :]),
        }
        for i in range(N_CORES)
    ]
    res = run_bass_kernel_spmd(nc, in_maps, core_ids=list(range(N_CORES))).results
    out = np.concatenate([res[i]["out"] for i in range(N_CORES)], axis=2)
    return out.astype(np.float32)


# revision 27
# speedup vs baseline: 1.0367x; 1.0367x over previous
"""CorrelationLayer1D Trainium2 Bass kernel (fp16 pipeline).

Computes out[b, d, h, w] = sum_c x_1[b,c,h,w] * x2p[b,c,h,w+d] for d in [0, 41),
where x2p is x_2 width-padded by (8, 32).  Inputs [4,128,160,320] f32.

Sharding: data-parallel over H = 160 = 8*20 (correlation runs along W only, so
H-sharding needs no halo).  Host converts inputs fp32 -> fp16 (rel-err budget
2e-2 >> fp16's ~1e-3) and converts the fp16 device output back to fp32.

Per core, rows are processed in chunks of HC=10 h-rows, software-pipelined
with lag 2 (the transpose/output stage of chunk N is emitted after the gram
stage of chunk N+2 so the DRAM round-trip latency hides under gram work):

Stage 1 (per chunk):
  x1/x2 load as contiguous [C, HC*W] fp16 tiles on the SP HWDGE ring, which
  carries nothing that waits on compute, so prefetch is never head-of-line
  blocked.  Per (h, w-block) the PE computes a block-diagonal Gram into
  PSUM: two M=64 matmuls (lo/hi) per 128-wide block land as a compacted
  [128, 104] band tile; 4 h-rows of one block share a PSUM bank
  ([128, 416] f32).  Edge blocks use narrowed windows (x2 is unpadded in
  SBUF) and the uncomputed strips are zeroed in the atlas by GpSimd.
  VectorE drains PSUM -> per-kblk fp16 atlas [128, HC*104].  GpSimd SWDGE
  stores the atlas to DRAM scratch (row pitch padded +63).

Stage 2 (lag 2 behind):
  Full-row skewed reloads (GpSimd SWDGE): partition i reads its whole
  HC*104 row shifted by i elements (DRAM-side AP partition step =
  pitch+1), so each partition is one contiguous descriptor and row h's
  41-wide band lands at [h*104, h*104+41).  SBUF-side per-partition skews
  are not expressible, which is why the round-trip exists.  PE transposes
  bands to [d, w] via matmul with an fp16 identity; rows h and h+HC/2
  share one PSUM bank at partition bases 0/64; ScalarE drains each
  [105, 320] bank into a [105, HC/2*W] tile whose halves store as two
  half-chunk output DMAs on the SP ring (the ACT HWDGE queue is served by
  a single SDMA engine, so bulk data never goes there).
"""
import sys

import numpy as np

try:
    import concourse.bass as bass  # noqa: F401
except ImportError:
    sys.path.insert(0, "/opt/trn_rl_repo")

import concourse.bass as bass
import concourse.tile as tile
from concourse import bacc, masks, mybir
from concourse.ap import AP
from concourse.bass_utils import run_bass_kernel_spmd

MAX_DISP = 40
D = MAX_DISP + 1  # 41 displacements
PAD_L = 8
B, C, H, W = 4, 128, 160, 320
N_CORES = 8
HS = H // N_CORES  # 20 h-rows per core
WBLOCKS = [(0, 128), (128, 128), (256, 64)]
GW = 64 + MAX_DISP  # compacted gram width per 64-block: 104

F32 = mybir.dt.float32
F16 = mybir.dt.float16

# (kblk, j0, N) for the lo-half matmul of each w-block: the window starts at
# x2p column w0+j0 and spans N columns, all of which exist in the unpadded x2
# row.  Gram columns outside [j0, j0+N) are zeroed in the atlas.
LO_WIN = {0: (8, 96), 1: (0, 104), 2: (0, 72)}


def build_kernel(b_dim=B, hs=HS, hc=None):
    if hc is None:
        hc = 10 if hs % 10 == 0 else hs
    assert hs % hc == 0 and hc % 2 == 0
    nchunks = hs // hc
    hgroups = [(s, min(4, hc - s)) for s in range(0, hc, 4)]

    nc = bacc.Bacc("TRN2", target_bir_lowering=False, debug=False)
    x1e = nc.declare_dram_parameter("x1", [b_dim, C, hs, W], F16, isOutput=False)
    x2e = nc.declare_dram_parameter("x2", [b_dim, C, hs, W], F16, isOutput=False)
    oute = nc.declare_dram_parameter("out", [b_dim, D, hs, W], F16, isOutput=True)

    with tile.TileContext(nc) as tc:
        with (
            tc.tile_pool(name="const", bufs=1) as const_pool,
            tc.tile_pool(name="xin", bufs=3) as xin_pool,
            tc.tile_pool(name="atlas", bufs=3) as atlas_pool,
            tc.tile_pool(name="sbig", bufs=3) as sbig_pool,
            tc.tile_pool(name="asm", bufs=3) as asm_pool,
            tc.tile_pool(name="psum_g", bufs=2, space="PSUM") as psum_g,
            tc.tile_pool(name="psum_t", bufs=2, space="PSUM") as psum_t,
            tc.tile_pool(name="scratch", bufs=3, space="DRAM") as scratch_pool,
        ):
            ident = const_pool.tile([128, 128], F16)
            masks.make_identity(nc, ident[:])

            def emit_stage1(b, ci):
                h0 = ci * hc
                x1b = xin_pool.tile([C, hc * W], F16, tag="x1b", name=f"x1b_{b}_{ci}")
                nc.sync.dma_start(
                    x1b[:].rearrange("p (h w) -> p h w", w=W),
                    x1e[b, :, h0 : h0 + hc, :],
                )
                x2b = xin_pool.tile([C, hc * W], F16, tag="x2b", name=f"x2b_{b}_{ci}")
                nc.sync.dma_start(
                    x2b[:].rearrange("p (h w) -> p h w", w=W),
                    x2e[b, :, h0 : h0 + hc, :],
                )

                atlas = [
                    atlas_pool.tile([128, hc * GW], F16, tag="atl0", name=f"atl0_{b}_{ci}"),
                    atlas_pool.tile([128, hc * GW], F16, tag="atl1", name=f"atl1_{b}_{ci}"),
                    atlas_pool.tile([64, hc * GW], F16, tag="atl2", name=f"atl2_{b}_{ci}"),
                ]

                # Gram phase: 4 h-rows of one kblk share a PSUM bank.
                ncopy = 0
                for s0, ng in hgroups:
                    pg = [
                        psum_g.tile([128, 4 * GW], F32, tag="pg0", name=f"pg0_{b}_{ci}_{s0}"),
                        psum_g.tile([128, 4 * GW], F32, tag="pg1", name=f"pg1_{b}_{ci}_{s0}"),
                        psum_g.tile([64, 4 * GW], F32, tag="pg2", name=f"pg2_{b}_{ci}_{s0}"),
                    ]
                    for hi in range(ng):
                        hh = s0 + hi
                        o1 = hh * W
                        for kblk, (w0, wb) in enumerate(WBLOCKS):
                            j0, nlo = LO_WIN[kblk]
                            c0 = hi * GW
                            nc.tensor.matmul(
                                pg[kblk][0:64, c0 + j0 : c0 + j0 + nlo],
                                x1b[:, o1 + w0 : o1 + w0 + 64],
                                x2b[:, o1 + w0 + j0 - PAD_L : o1 + w0 + j0 - PAD_L + nlo],
                                start=True,
                                stop=True,
                                tile_position=(0, 0),
                            )
                            if wb == 128:
                                nc.tensor.matmul(
                                    pg[kblk][64:128, c0 : c0 + GW],
                                    x1b[:, o1 + w0 + 64 : o1 + w0 + 128],
                                    x2b[:, o1 + w0 + 56 : o1 + w0 + 56 + GW],
                                    start=True,
                                    stop=True,
                                    tile_position=(0, 64),
                                )
                    # Drain the 3 banks to the fp16 atlas (DVE only: the ACT
                    # and SP sequencers stay free to issue DMAs without
                    # queueing copies behind semaphore waits).
                    for kblk, (w0, wb) in enumerate(WBLOCKS):
                        dst = atlas[kblk][0:wb, s0 * GW : (s0 + ng) * GW]
                        srcp = pg[kblk][0:wb, 0 : ng * GW]
                        nc.vector.tensor_copy(dst, srcp)
                        ncopy += 1

                # Zero the gram strips the narrowed edge matmuls skipped.
                # On DVE: they serialize after the atlas copies anyway (WAW),
                # so same-queue ordering is free and the GpSimd store issues
                # without an extra cross-engine semaphore hop.
                a0 = atlas[0][0:64, :].rearrange("p (h j) -> p h j", j=GW)
                nc.vector.memset(a0[:, :, 0:8], 0.0)
                a2 = atlas[2][0:64, :].rearrange("p (h j) -> p h j", j=GW)
                nc.vector.memset(a2[:, :, 72:GW], 0.0)

                # Scratch round-trip: rectangular store (row pitch padded by
                # 63 so the skewed read stays in-bounds), then a full-row
                # skewed reload: partition i reads the whole hc*104 row
                # shifted by i elements, so each partition is one contiguous
                # descriptor and the band of row h sits at [h*104, h*104+41).
                RW = hc * GW
                R = RW + 63
                sbig = [
                    sbig_pool.tile([128, RW], F16, tag="sb0", name=f"sb0_{b}_{ci}"),
                    sbig_pool.tile([128, RW], F16, tag="sb1", name=f"sb1_{b}_{ci}"),
                    sbig_pool.tile([64, RW], F16, tag="sb2", name=f"sb2_{b}_{ci}"),
                ]
                scrs = []
                for kblk, (w0, wb) in enumerate(WBLOCKS):
                    scr = scratch_pool.tile(
                        [wb, R], F16, tag=f"scr{kblk}", name=f"scr{kblk}_{b}_{ci}"
                    )
                    nc.gpsimd.dma_start(scr[:, 0:RW], atlas[kblk][0:wb, :])
                    scrs.append(scr)
                return b, ci, sbig, scrs

            def emit_stage2(state):
                b, ci, sbig, scrs = state
                h0 = ci * hc
                hc2 = hc // 2
                RW = hc * GW
                R = RW + 63
                # Full-row skewed reloads: partition step = pitch+1.
                for kblk, (w0, wb) in enumerate(WBLOCKS):
                    scr_ap = scrs[kblk][:]
                    for half in range(wb // 64):
                        diag = AP(
                            tensor=scr_ap.tensor,
                            offset=scr_ap.offset + half * 64 * R,
                            ap=[[R + 1, 64], [1, RW]],
                        )
                        nc.gpsimd.dma_start(
                            sbig[kblk][half * 64 : (half + 1) * 64, :], diag
                        )
                # Transpose bands to [d, w]: rows h and h+hc/2 share one PSUM
                # bank at partition bases 0 and 64, drained by a single copy
                # into a [105, hc/2*W] tile whose halves store as two
                # contiguous half-chunk output DMAs.
                abatch = asm_pool.tile(
                    [64 + D, hc2 * W], F16, tag="ab", name=f"ab_{b}_{ci}"
                )
                for p0 in range(hc2):
                    pt = psum_t.tile(
                        [64 + D, W], F32, tag="pt", name=f"pt_{b}_{ci}_{p0}"
                    )
                    for kblk, (w0, wb) in enumerate(WBLOCKS):
                        nc.tensor.matmul(
                            pt[0:D, w0 : w0 + wb],
                            sbig[kblk][0:wb, p0 * GW : p0 * GW + D],
                            ident[0:wb, 0:wb],
                            start=True,
                            stop=True,
                            tile_position=(0, 0),
                        )
                        nc.tensor.matmul(
                            pt[64 : 64 + D, w0 : w0 + wb],
                            sbig[kblk][0:wb, (p0 + hc2) * GW : (p0 + hc2) * GW + D],
                            ident[0:wb, 0:wb],
                            start=True,
                            stop=True,
                            tile_position=(0, 64),
                        )
                    nc.scalar.copy(abatch[:, p0 * W : (p0 + 1) * W], pt[:])

                nc.sync.dma_start(
                    oute[b, :, h0 : h0 + hc2, :],
                    abatch[0:D, :].rearrange("d (h w) -> d h w", w=W),
                )
                nc.sync.dma_start(
                    oute[b, :, h0 + hc2 : h0 + hc, :],
                    abatch[64 : 64 + D, :].rearrange("d (h w) -> d h w", w=W),
                )

            # Software pipeline, lag 2: stage2 of chunk N runs after stage1
            # of chunk N+2, so the ~13us scratch round-trip latency of chunk
            # N hides under two chunks of gram work.
            LAG = 2
            chunks = [(b, ci) for b in range(b_dim) for ci in range(nchunks)]
            pending = []
            for b, ci in chunks:
                pending.append(emit_stage1(b, ci))
                if len(pending) > LAG:
                    emit_stage2(pending.pop(0))
            for st in pending:
                emit_stage2(st)

    nc.finalize()
    return nc


_compiled = {}


def _get_kernel(b_dim, hs):
    key = (b_dim, hs)
    if key not in _compiled:
        _compiled[key] = build_kernel(b_dim, hs)
    return _compiled[key]


def kernel(x_1: np.ndarray, x_2: np.ndarray) -> np.ndarray:
    assert x_1.shape == (B, C, H, W) and x_2.shape == (B, C, H, W)
    x_1 = np.ascontiguousarray(x_1, dtype=np.float16)
    x_2 = np.ascontiguousarray(x_2, dtype=np.float16)
    nc = _get_kernel(B, HS)
    in_maps = [
        {
            "x1": np.ascontiguousarray(x_1[:, :, i * HS : (i + 1) * HS, :]),
            "x2": np.ascontiguousarray(x_2[:, :, i * HS : (i + 1) * HS, :]),
        }
        for i in range(N_CORES)
    ]
    res = run_bass_kernel_spmd(nc, in_maps, core_ids=list(range(N_CORES))).results
    out = np.concatenate([res[i]["out"] for i in range(N_CORES)], axis=2)
    return out.astype(np.float32)
